# revision 7
# baseline (speedup 1.0000x reference)
"""CRF cell (Viterbi decode + forward log-partition + gold-path score) on 8
Trainium NeuronCores.

Problem: B=512, T=1024, K=64; mask is all-ones (per input spec).
Output: (decode int32 [B,T], loss f32 [B]).

Strategy (data-parallel over B, 64 sequences per core):
  Viterbi  — per step, the exact 64-way max over predecessors is ONE DVE
             tensor_tensor_scan over free dim (j,i) with a precomputed
             delta table:  g[j,i] = max(g[j,i-1] + (t[i-1,j]-t[i,j]), S[i])
             so g[j,63] + t[63,j] = max_i(S[i] + t[i,j]).  delta[j,0]=-1e9
             resets the chain at each j group.  Scores (renormalized by the
             per-sequence max) stream to DRAM; the host recovers the argmax
             backpointers only along the decoded paths.
  Forward  — scaled-probability domain on the TensorEngine:
             P = [exp(trans) | ones]^T @ u  (PSUM row 64 = column sum s),
             u <- P[0:64] * exp(em_t), divided by a lagged column sum every
             DIV steps; the s values stream out and the host sums their logs.
  Numerator— O(B*T) gathers, done on host.
"""
import os
import sys
import numpy as np

sys.path.insert(0, "/opt/trn_rl_repo")

B_FULL, T, K = 512, 1024, 64
NCORES = 8
B = B_FULL // NCORES          # 64 sequences per core
CHUNK = 16                    # time steps per emissions DMA
DIV = 4                       # forward renormalization cadence
ND = (T - 1) // DIV           # number of division points (t = DIV, 2*DIV, ...)
JI = K * K                    # viterbi scan free size

_CACHED = {}


def _build_nc():
    import concourse.bass as bass
    import concourse.bacc as bacc
    import concourse.mybir as mybir
    from concourse.tile import TileContext

    fp = mybir.dt.float32
    nc = bacc.Bacc("TRN2", target_bir_lowering=False, debug=False,
                   num_devices=NCORES)

    NCH = T // CHUNK
    # inputs (per core)
    s0 = nc.declare_dram_parameter("s0", [B, K], fp, isOutput=False)
    u0 = nc.declare_dram_parameter("u0", [K, B], fp, isOutput=False)
    delta = nc.declare_dram_parameter("delta", [B, JI], fp, isOutput=False)
    ew = nc.declare_dram_parameter("ew", [K, K + 1], fp, isOutput=False)
    ones_r = nc.declare_dram_parameter("ones_r", [1, K], fp, isOutput=False)
    # emissions, b-major with trans[63,:] pre-added: em_b[ci, b, (tt,k)]
    em_b = nc.declare_dram_parameter("em_b", [NCH, B, CHUNK * K], fp,
                                     isOutput=False)
    # emissions, k-major raw: em_k[ci, j, (tt,b)]
    em_k = nc.declare_dram_parameter("em_k", [NCH, K, CHUNK * B], fp,
                                     isOutput=False)
    # outputs
    s_traj = nc.declare_dram_parameter("s_traj", [T - 1, B, K], fp,
                                       isOutput=True)
    ufin = nc.declare_dram_parameter("ufin", [K, B], fp, isOutput=True)
    s_ship = nc.declare_dram_parameter("s_ship", [1, ND * K], fp,
                                       isOutput=True)

    with TileContext(nc) as tc:
        with (
            tc.tile_pool(name="const", bufs=1) as cpool,
            tc.tile_pool(name="state", bufs=1) as spool,
            tc.tile_pool(name="g", bufs=2) as gpool,
            tc.tile_pool(name="emb", bufs=2) as embpool,
            tc.tile_pool(name="emk", bufs=2) as emkpool,
            tc.tile_pool(name="fw", bufs=2) as fwpool,
            tc.tile_pool(name="psum", bufs=2, space="PSUM") as ppool,
            tc.tile_pool(name="psum1", bufs=1, space="PSUM") as ppool1,
        ):
            # ---- constants ----
            dtile = cpool.tile([B, JI], fp, tag="delta")
            nc.sync.dma_start(out=dtile[:], in_=delta[:])
            ewt = cpool.tile([K, K + 1], fp, tag="ew")
            nc.sync.dma_start(out=ewt[:], in_=ew[:])
            onesr = cpool.tile([1, K], fp, tag="ones")
            nc.sync.dma_start(out=onesr[:], in_=ones_r[:])

            # ---- state ----
            sA = spool.tile([B, K], fp, tag="sA")
            sB = spool.tile([B, K], fp, tag="sB")
            nc.sync.dma_start(out=sA[:], in_=s0[:])
            uA = spool.tile([K, B], fp, tag="uA")
            uB = spool.tile([K, B], fp, tag="uB")
            nc.sync.dma_start(out=uA[:], in_=u0[:])
            top1 = spool.tile([B, 1], fp, tag="top1")
            rinv = spool.tile([1, K], fp, tag="rinv")
            sshipt = spool.tile([1, ND * K], fp, tag="sship")

            # absorb multi-queue DMA waits into single vector ops
            junk = spool.tile([B, 1], fp, tag="junk")
            nc.vector.tensor_copy(junk[:1], dtile[:1, 0:1])
            nc.vector.tensor_copy(junk[:1], ewt[:1, 0:1])
            nc.vector.tensor_copy(junk[:1], sA[:1, 0:1])
            nc.vector.tensor_copy(junk[:1], uA[:1, 0:1])
            nc.vector.tensor_reduce(out=top1[:], in_=sA[:],
                                    axis=mybir.AxisListType.X,
                                    op=mybir.AluOpType.max)

            rinv_rep = None   # PSUM tile holding rinv broadcast across partitions

            for t in range(1, T):
                ci, tt = (t // CHUNK), (t % CHUNK)
                cur, nxt = (sA, sB) if t % 2 == 1 else (sB, sA)
                ucur, unxt = (uA, uB) if t % 2 == 1 else (uB, uA)
                if tt == 0 or t == 1:
                    embt = embpool.tile([B, CHUNK * K], fp, tag="emb")
                    nc.sync.dma_start(out=embt[:], in_=em_b[ci])
                    nc.vector.tensor_copy(junk[:1], embt[:1, 0:1])
                    emkt = emkpool.tile([K, CHUNK * B], fp, tag="emk")
                    nc.sync.dma_start(out=emkt[:], in_=em_k[ci])
                    nc.vector.tensor_copy(junk[:1], emkt[:1, 0:1])

                # ---------- viterbi ----------
                g = gpool.tile([B, JI], fp, tag="g")
                g3 = bass.AP(g.tensor, g[:].offset, [g[:].ap[0], [K, K], [1, K]])
                d3 = bass.AP(dtile.tensor, dtile[:].offset,
                             [dtile[:].ap[0], [K, K], [1, K]])
                prev_b = bass.AP(cur.tensor, cur[:].offset,
                                 [cur[:].ap[0], [0, K], [1, K]])
                nc.vector.add_instruction(
                    mybir.InstTensorScalarPtr(
                        name=nc.get_next_instruction_name(),
                        is_tensor_tensor_scan=True,
                        is_scalar_tensor_tensor=True,
                        op0=mybir.AluOpType.add,
                        op1=mybir.AluOpType.max,
                        ins=[
                            nc.vector.lower_ap(d3),
                            nc.vector.lower_ap_or_imm(-1e9),
                            nc.vector.lower_ap(prev_b),
                        ],
                        outs=[nc.vector.lower_ap(g3)],
                    ))
                # S = (g[:, j, 63] - top1) + em'   (em' has trans[63,:] folded in)
                g63 = bass.AP(g.tensor, g[:].offset + (K - 1),
                              [g[:].ap[0], [K, K]])
                nc.vector.scalar_tensor_tensor(
                    out=nxt[:], in0=g63, scalar=top1[:],
                    in1=embt[:, tt * K:(tt + 1) * K],
                    op0=mybir.AluOpType.subtract, op1=mybir.AluOpType.add)
                nc.vector.tensor_reduce(
                    out=top1[:], in_=nxt[:], axis=mybir.AxisListType.X,
                    op=mybir.AluOpType.max)
                nc.sync.dma_start(out=s_traj[t - 1], in_=nxt[:])

                # ---------- forward ----------
                P = ppool.tile([K + 1, B], fp, tag="P")
                nc.tensor.matmul(P[:], ewt[:], ucur[:])
                expem = fwpool.tile([K, B], fp, tag="expem")
                nc.scalar.activation(
                    out=expem[:], in_=emkt[:, tt * B:(tt + 1) * B],
                    func=mybir.ActivationFunctionType.Exp)
                if t % DIV == 0:
                    d = t // DIV   # 1-based division index
                    # ship s-row; build rinv from it; broadcast via PE
                    nc.scalar.copy(out=sshipt[:, (d - 1) * K:d * K],
                                   in_=P[K:K + 1, :])
                    nc.vector.reciprocal(out=rinv[:], in_=P[K:K + 1, :])
                    rinv_rep = ppool1.tile([K, B], fp, tag="rrep")
                    nc.tensor.matmul(rinv_rep[:], onesr[:], rinv[:])
                    tmpu = fwpool.tile([K, B], fp, tag="tmpu")
                    nc.vector.tensor_tensor(
                        out=tmpu[:], in0=P[0:K, :], in1=expem[:],
                        op=mybir.AluOpType.mult)
                    nc.vector.tensor_tensor(
                        out=unxt[:], in0=tmpu[:], in1=rinv_rep[:],
                        op=mybir.AluOpType.mult)
                else:
                    nc.vector.tensor_tensor(
                        out=unxt[:], in0=P[0:K, :], in1=expem[:],
                        op=mybir.AluOpType.mult)

            last_u = uA if (T - 1) % 2 == 0 else uB
            nc.sync.dma_start(out=ufin[:], in_=last_u[:])
            nc.sync.dma_start(out=s_ship[:], in_=sshipt[:])

    nc.compile()
    return nc


def _prep_core_inputs(emissions, start, trans):
    """Per-core input dict list. emissions [B_FULL,T,K] f32."""
    NCH = T // CHUNK
    expT = np.exp(trans).astype(np.float32)
    ew = np.concatenate([expT, np.ones((K, 1), np.float32)], axis=1)  # [K,K+1]
    delta = np.empty((K, K), np.float32)          # [j, i]
    delta[:, 0] = -1e9
    delta[:, 1:] = (trans[:-1, :] - trans[1:, :]).T
    delta_b = np.broadcast_to(delta.reshape(1, JI), (B, JI)).copy()
    ones_r = np.ones((1, K), np.float32)
    t63 = trans[K - 1]                            # [K] folded into em_b

    in_maps = []
    for c in range(NCORES):
        emc = emissions[c * B:(c + 1) * B]        # [B,T,K]
        s0 = (start[None, :] + emc[:, 0, :]).astype(np.float32)
        u0 = np.exp(s0).T.copy()                  # [K,B]
        # em_b: steps 1..T-1 used; keep full T for simple indexing
        emb = (emc + t63[None, None, :]).astype(np.float32)       # [B,T,K]
        emb = emb.transpose(1, 0, 2).reshape(NCH, CHUNK, B, K) \
            .transpose(0, 2, 1, 3).reshape(NCH, B, CHUNK * K).copy()
        emk = emc.transpose(1, 2, 0).reshape(NCH, CHUNK, K, B) \
            .transpose(0, 2, 1, 3).reshape(NCH, K, CHUNK * B).copy()
        in_maps.append({"s0": s0, "u0": u0, "delta": delta_b, "ew": ew,
                        "ones_r": ones_r, "em_b": emb, "em_k": emk})
    return in_maps


def _host_postprocess(results, emissions, tags, start, end, trans):
    """Backtrack + numerator + loss from device outputs."""
    Bf = B_FULL
    # gather trajectories: S[t] for t=1..T-1 per core -> [T-1, B_FULL, K]
    straj = np.concatenate([r["s_traj"] for r in results], axis=1)
    s0_full = (start[None, :] + emissions[:, 0, :]).astype(np.float32)

    # ---- viterbi backtrack ----
    transT = np.ascontiguousarray(trans.T)        # [j, i]
    last = np.argmax(straj[T - 2] + end[None, :], axis=1).astype(np.int32)
    decode = np.empty((Bf, T), np.int32)
    decode[:, T - 1] = last
    idx = last
    for t in range(T - 2, -1, -1):
        Sprev = straj[t - 1] if t >= 1 else s0_full
        idx = np.argmax(Sprev + transT[idx], axis=1).astype(np.int32)
        decode[:, t] = idx

    # ---- logZ ----
    ufin = np.concatenate([r["ufin"] for r in results], axis=1)   # [K, B_FULL]
    # s_ship[c] is [ND, B] (B == K == 64): per-division column sums per seq
    logacc = np.concatenate(
        [np.log(r["s_ship"].reshape(ND, B)).sum(axis=0) for r in results])
    z_tail = (np.exp(end)[:, None] * ufin).sum(axis=0)            # [B_FULL]
    logz = logacc + np.log(z_tail)

    # ---- numerator (gold path score), mask all ones ----
    tg = tags.astype(np.int64)
    em_sc = np.take_along_axis(emissions, tg[:, :, None], axis=2)[:, :, 0]
    trans_sc = trans[tg[:, :-1], tg[:, 1:]]
    num = (start[tg[:, 0]] + em_sc.sum(axis=1) + trans_sc.sum(axis=1)
           + end[tg[:, -1]])

    loss = -(num - logz)
    return decode, loss.astype(np.float32)


def _run_device(in_maps):
    from concourse.bass_utils import run_bass_kernel_spmd
    if "nc" not in _CACHED:
        _CACHED["nc"] = _build_nc()
    res = run_bass_kernel_spmd(_CACHED["nc"], in_maps, list(range(NCORES)))
    return res.results


def kernel(emissions, attn_mask, tags, start_transitions, end_transitions,
           transitions):
    emissions = np.asarray(emissions, np.float32)
    tags = np.asarray(tags)
    start = np.asarray(start_transitions, np.float32)
    end = np.asarray(end_transitions, np.float32)
    trans = np.asarray(transitions, np.float32)
    # attn_mask is all ones per the input spec; the kernel relies on that.

    in_maps = _prep_core_inputs(emissions, start, trans)
    results = _run_device(in_maps)
    decode, loss = _host_postprocess(results, emissions, tags, start, end,
                                     trans)
    return decode, loss


# ---------------------------------------------------------------------------
# numpy replica of the device program, for validation without hardware
def _device_replica(in_map):
    s0 = in_map["s0"]; u0 = in_map["u0"]
    ew = in_map["ew"]; em_b = in_map["em_b"]; em_k = in_map["em_k"]
    NCH = T // CHUNK
    emb = em_b.reshape(NCH, B, CHUNK, K).transpose(0, 2, 1, 3) \
        .reshape(T, B, K)
    emk = em_k.reshape(NCH, K, CHUNK, B).transpose(0, 2, 1, 3) \
        .reshape(T, K, B)
    S = s0.copy(); u = u0.copy()
    top1 = S.max(1, keepdims=True)
    straj = np.zeros((T - 1, B, K), np.float32)
    sship = np.zeros((ND, K), np.float32)
    trans = in_map["_trans"]
    for t in range(1, T):
        # viterbi max (direct; the chained-delta fp path differs by ~3e-6)
        M = (S[:, :, None] + trans[None]).max(1)        # [B, K(j)]
        S = (M - trans[K - 1][None, :] - top1) + emb[t]
        top1 = S.max(1, keepdims=True)
        straj[t - 1] = S
        # forward replica
        P = ew.T @ u                       # [K+1, B]
        expem = np.exp(emk[t])
        if t % DIV == 0:
            d = t // DIV
            sship[d - 1] = P[K]
            rinv = 1.0 / P[K]
            u = (P[0:K] * expem) * rinv[None, :]
        else:
            u = P[0:K] * expem
    return {"s_traj": straj, "ufin": u, "s_ship": sship.reshape(1, ND * K)}


def _kernel_replica(emissions, attn_mask, tags, start_transitions,
                    end_transitions, transitions):
    """Full-numpy emulation of kernel() — for numerics validation."""
    emissions = np.asarray(emissions, np.float32)
    start = np.asarray(start_transitions, np.float32)
    end = np.asarray(end_transitions, np.float32)
    trans = np.asarray(transitions, np.float32)
    in_maps = _prep_core_inputs(emissions, start, trans)
    for m in in_maps:
        m["_trans"] = trans
    results = [_device_replica(m) for m in in_maps]
    return _host_postprocess(results, emissions, np.asarray(tags), start, end,
                             trans)


# revision 8
# speedup vs baseline: 95.0957x; 95.0957x over previous
"""CRF cell (Viterbi decode + forward log-partition + gold-path score) on 8
Trainium NeuronCores.

Problem: B=512, T=1024, K=64; mask is all-ones (per input spec).
Output: (decode int32 [B,T], loss f32 [B]).

Strategy (data-parallel over B, 64 sequences per core):
  Viterbi  — per step, the exact 64-way max over predecessors is ONE DVE
             tensor_tensor_scan over free dim (j,i) with a precomputed
             delta table:  g[j,i] = max(g[j,i-1] + (t[i-1,j]-t[i,j]), S[i])
             so g[j,63] + t[63,j] = max_i(S[i] + t[i,j]).  delta[j,0]=-1e9
             resets the chain at each j group.  Scores (renormalized by the
             per-sequence max) stream to DRAM; the host recovers the argmax
             backpointers only along the decoded paths.
  Forward  — scaled-probability domain on the TensorEngine:
             P = [exp(trans) | ones]^T @ u  (PSUM row 64 = column sum s),
             u <- P[0:64] * exp(em_t), divided by a lagged column sum every
             DIV steps; the s values stream out and the host sums their logs.
  Numerator— O(B*T) gathers, done on host.
"""
import os
import sys
import numpy as np

sys.path.insert(0, "/opt/trn_rl_repo")

B_FULL, T, K = 512, 1024, 64
NCORES = 8
B = B_FULL // NCORES          # 64 sequences per core
CHUNK = 16                    # time steps per emissions DMA
DIV = 4                       # forward renormalization cadence
ND = (T - 1) // DIV           # number of division points (t = DIV, 2*DIV, ...)
JI = K * K                    # viterbi scan free size

_CACHED = {}


def _build_nc():
    import concourse.bass as bass
    import concourse.bacc as bacc
    import concourse.mybir as mybir
    from concourse.tile import TileContext

    fp = mybir.dt.float32
    nc = bacc.Bacc("TRN2", target_bir_lowering=False, debug=False,
                   num_devices=NCORES)

    NCH = T // CHUNK
    # inputs (per core)
    s0 = nc.declare_dram_parameter("s0", [B, K], fp, isOutput=False)
    u0 = nc.declare_dram_parameter("u0", [K, B], fp, isOutput=False)
    delta = nc.declare_dram_parameter("delta", [B, JI], fp, isOutput=False)
    ew = nc.declare_dram_parameter("ew", [K, K + 1], fp, isOutput=False)
    ones_r = nc.declare_dram_parameter("ones_r", [1, K], fp, isOutput=False)
    # emissions, b-major with trans[63,:] pre-added: em_b[ci, b, (tt,k)]
    em_b = nc.declare_dram_parameter("em_b", [NCH, B, CHUNK * K], fp,
                                     isOutput=False)
    # emissions, k-major raw: em_k[ci, j, (tt,b)]
    em_k = nc.declare_dram_parameter("em_k", [NCH, K, CHUNK * B], fp,
                                     isOutput=False)
    # outputs
    s_traj = nc.declare_dram_parameter("s_traj", [T - 1, B, K], fp,
                                       isOutput=True)
    ufin = nc.declare_dram_parameter("ufin", [K, B], fp, isOutput=True)
    s_ship = nc.declare_dram_parameter("s_ship", [1, ND * K], fp,
                                       isOutput=True)

    with TileContext(nc) as tc:
        with (
            tc.tile_pool(name="const", bufs=1) as cpool,
            tc.tile_pool(name="state", bufs=1) as spool,
            tc.tile_pool(name="g", bufs=2) as gpool,
            tc.tile_pool(name="emb", bufs=2) as embpool,
            tc.tile_pool(name="emk", bufs=2) as emkpool,
            tc.tile_pool(name="fw", bufs=2) as fwpool,
            tc.tile_pool(name="psum", bufs=2, space="PSUM") as ppool,
            tc.tile_pool(name="psum1", bufs=1, space="PSUM") as ppool1,
        ):
            # ---- constants ----
            dtile = cpool.tile([B, JI], fp, tag="delta")
            nc.sync.dma_start(out=dtile[:], in_=delta[:])
            ewt = cpool.tile([K, K + 1], fp, tag="ew")
            nc.sync.dma_start(out=ewt[:], in_=ew[:])
            onesr = cpool.tile([1, K], fp, tag="ones")
            nc.sync.dma_start(out=onesr[:], in_=ones_r[:])

            # ---- state ----
            sA = spool.tile([B, K], fp, tag="sA")
            sB = spool.tile([B, K], fp, tag="sB")
            nc.sync.dma_start(out=sA[:], in_=s0[:])
            uA = spool.tile([K, B], fp, tag="uA")
            uB = spool.tile([K, B], fp, tag="uB")
            nc.sync.dma_start(out=uA[:], in_=u0[:])
            top1 = spool.tile([B, 1], fp, tag="top1")
            rinv = spool.tile([1, K], fp, tag="rinv")
            sshipt = spool.tile([1, ND * K], fp, tag="sship")

            # absorb multi-queue DMA waits into single vector ops
            junk = spool.tile([B, 1], fp, tag="junk")
            nc.vector.tensor_copy(junk[:1], dtile[:1, 0:1])
            nc.vector.tensor_copy(junk[:1], ewt[:1, 0:1])
            nc.vector.tensor_copy(junk[:1], sA[:1, 0:1])
            nc.vector.tensor_copy(junk[:1], uA[:1, 0:1])
            nc.vector.tensor_reduce(out=top1[:], in_=sA[:],
                                    axis=mybir.AxisListType.X,
                                    op=mybir.AluOpType.max)

            rinv_rep = None   # PSUM tile holding rinv broadcast across partitions

            for t in range(1, T):
                ci, tt = (t // CHUNK), (t % CHUNK)
                cur, nxt = (sA, sB) if t % 2 == 1 else (sB, sA)
                ucur, unxt = (uA, uB) if t % 2 == 1 else (uB, uA)
                if tt == 0 or t == 1:
                    embt = embpool.tile([B, CHUNK * K], fp, tag="emb")
                    nc.sync.dma_start(out=embt[:], in_=em_b[ci])
                    nc.vector.tensor_copy(junk[:1], embt[:1, 0:1])
                    emkt = emkpool.tile([K, CHUNK * B], fp, tag="emk")
                    nc.sync.dma_start(out=emkt[:], in_=em_k[ci])
                    nc.vector.tensor_copy(junk[:1], emkt[:1, 0:1])

                # ---------- viterbi ----------
                g = gpool.tile([B, JI], fp, tag="g")
                g3 = bass.AP(g.tensor, g[:].offset, [g[:].ap[0], [K, K], [1, K]])
                d3 = bass.AP(dtile.tensor, dtile[:].offset,
                             [dtile[:].ap[0], [K, K], [1, K]])
                prev_b = bass.AP(cur.tensor, cur[:].offset,
                                 [cur[:].ap[0], [0, K], [1, K]])
                nc.vector.add_instruction(
                    mybir.InstTensorScalarPtr(
                        name=nc.get_next_instruction_name(),
                        is_tensor_tensor_scan=True,
                        is_scalar_tensor_tensor=True,
                        op0=mybir.AluOpType.add,
                        op1=mybir.AluOpType.max,
                        ins=[
                            nc.vector.lower_ap(d3),
                            nc.vector.lower_ap_or_imm(-1e9),
                            nc.vector.lower_ap(prev_b),
                        ],
                        outs=[nc.vector.lower_ap(g3)],
                    ))
                # S = (g[:, j, 63] - top1) + em'   (em' has trans[63,:] folded in)
                g63 = bass.AP(g.tensor, g[:].offset + (K - 1),
                              [g[:].ap[0], [K, K]])
                nc.vector.scalar_tensor_tensor(
                    out=nxt[:], in0=g63, scalar=top1[:],
                    in1=embt[:, tt * K:(tt + 1) * K],
                    op0=mybir.AluOpType.subtract, op1=mybir.AluOpType.add)
                nc.vector.tensor_reduce(
                    out=top1[:], in_=nxt[:], axis=mybir.AxisListType.X,
                    op=mybir.AluOpType.max)
                nc.sync.dma_start(out=s_traj[t - 1], in_=nxt[:])

                # ---------- forward ----------
                P = ppool.tile([K + 1, B], fp, tag="P")
                nc.tensor.matmul(P[:], ewt[:], ucur[:])
                expem = fwpool.tile([K, B], fp, tag="expem")
                nc.scalar.activation(
                    out=expem[:], in_=emkt[:, tt * B:(tt + 1) * B],
                    func=mybir.ActivationFunctionType.Exp)
                if t % DIV == 0:
                    d = t // DIV   # 1-based division index
                    # ship s-row; build rinv from it; broadcast via PE
                    nc.scalar.copy(out=sshipt[:, (d - 1) * K:d * K],
                                   in_=P[K:K + 1, :])
                    nc.vector.reciprocal(out=rinv[:], in_=P[K:K + 1, :])
                    rinv_rep = ppool1.tile([K, B], fp, tag="rrep")
                    nc.tensor.matmul(rinv_rep[:], onesr[:], rinv[:])
                    tmpu = fwpool.tile([K, B], fp, tag="tmpu")
                    nc.vector.tensor_tensor(
                        out=tmpu[:], in0=P[0:K, :], in1=expem[:],
                        op=mybir.AluOpType.mult)
                    nc.vector.tensor_tensor(
                        out=unxt[:], in0=tmpu[:], in1=rinv_rep[:],
                        op=mybir.AluOpType.mult)
                else:
                    nc.vector.tensor_tensor(
                        out=unxt[:], in0=P[0:K, :], in1=expem[:],
                        op=mybir.AluOpType.mult)

            last_u = uA if (T - 1) % 2 == 0 else uB
            nc.sync.dma_start(out=ufin[:], in_=last_u[:])
            nc.sync.dma_start(out=s_ship[:], in_=sshipt[:])

    nc.compile()
    return nc


def _prep_core_inputs(emissions, start, trans):
    """Per-core input dict list. emissions [B_FULL,T,K] f32."""
    NCH = T // CHUNK
    expT = np.exp(trans).astype(np.float32)
    ew = np.concatenate([expT, np.ones((K, 1), np.float32)], axis=1)  # [K,K+1]
    delta = np.empty((K, K), np.float32)          # [j, i]
    delta[:, 0] = -1e9
    delta[:, 1:] = (trans[:-1, :] - trans[1:, :]).T
    delta_b = np.broadcast_to(delta.reshape(1, JI), (B, JI)).copy()
    ones_r = np.ones((1, K), np.float32)
    t63 = trans[K - 1]                            # [K] folded into em_b

    in_maps = []
    for c in range(NCORES):
        emc = emissions[c * B:(c + 1) * B]        # [B,T,K]
        s0 = (start[None, :] + emc[:, 0, :]).astype(np.float32)
        u0 = np.exp(s0).T.copy()                  # [K,B]
        # em_b: steps 1..T-1 used; keep full T for simple indexing
        emb = (emc + t63[None, None, :]).astype(np.float32)       # [B,T,K]
        emb = emb.transpose(1, 0, 2).reshape(NCH, CHUNK, B, K) \
            .transpose(0, 2, 1, 3).reshape(NCH, B, CHUNK * K).copy()
        emk = emc.transpose(1, 2, 0).reshape(NCH, CHUNK, K, B) \
            .transpose(0, 2, 1, 3).reshape(NCH, K, CHUNK * B).copy()
        in_maps.append({"s0": s0, "u0": u0, "delta": delta_b, "ew": ew,
                        "ones_r": ones_r, "em_b": emb, "em_k": emk})
    return in_maps


def _host_postprocess(results, emissions, tags, start, end, trans):
    """Backtrack + numerator + loss from device outputs."""
    Bf = B_FULL
    # gather trajectories: S[t] for t=1..T-1 per core -> [T-1, B_FULL, K]
    straj = np.concatenate([r["s_traj"] for r in results], axis=1)
    s0_full = (start[None, :] + emissions[:, 0, :]).astype(np.float32)

    # ---- viterbi backtrack ----
    transT = np.ascontiguousarray(trans.T)        # [j, i]
    last = np.argmax(straj[T - 2] + end[None, :], axis=1).astype(np.int32)
    decode = np.empty((Bf, T), np.int32)
    decode[:, T - 1] = last
    idx = last
    for t in range(T - 2, -1, -1):
        Sprev = straj[t - 1] if t >= 1 else s0_full
        idx = np.argmax(Sprev + transT[idx], axis=1).astype(np.int32)
        decode[:, t] = idx

    # ---- logZ ----
    ufin = np.concatenate([r["ufin"] for r in results], axis=1)   # [K, B_FULL]
    # s_ship[c] is [ND, B] (B == K == 64): per-division column sums per seq
    logacc = np.concatenate(
        [np.log(r["s_ship"].reshape(ND, B)).sum(axis=0) for r in results])
    z_tail = (np.exp(end)[:, None] * ufin).sum(axis=0)            # [B_FULL]
    logz = logacc + np.log(z_tail)

    # ---- numerator (gold path score), mask all ones ----
    tg = tags.astype(np.int64)
    em_sc = np.take_along_axis(emissions, tg[:, :, None], axis=2)[:, :, 0]
    trans_sc = trans[tg[:, :-1], tg[:, 1:]]
    num = (start[tg[:, 0]] + em_sc.sum(axis=1) + trans_sc.sum(axis=1)
           + end[tg[:, -1]])

    loss = -(num - logz)
    return decode, loss.astype(np.float32)


def _get_runner():
    """Build + jit the SPMD executable once; returns a callable over in_maps.

    Modeled on bass2jax.run_bass_via_pjrt, but caches the jitted function so
    repeat calls skip retrace/recompile, and exposes device-level run for
    timing.
    """
    if "runner" in _CACHED:
        return _CACHED["runner"]
    import jax
    import numpy as np_
    from jax.sharding import Mesh, PartitionSpec
    from jax.experimental.shard_map import shard_map
    import concourse.mybir as mybir
    from concourse import bass2jax

    nc = _build_nc()
    bass2jax.install_neuronx_cc_hook()
    partition_name = (nc.partition_id_tensor.name
                      if nc.partition_id_tensor else None)

    in_names, out_names, out_avals, zero_outs = [], [], [], []
    for alloc in nc.m.functions[0].allocations:
        if not isinstance(alloc, mybir.MemoryLocationSet):
            continue
        name = alloc.memorylocations[0].name
        if alloc.kind == "ExternalInput":
            if name != partition_name:
                in_names.append(name)
        elif alloc.kind == "ExternalOutput":
            out_names.append(name)
            shape = tuple(alloc.tensor_shape)
            dtype = mybir.dt.np(alloc.dtype)
            out_avals.append(jax.core.ShapedArray(shape, dtype))
            zero_outs.append(np_.zeros(shape, dtype))
    n_params = len(in_names)
    all_in_names = list(in_names) + list(out_names)
    if partition_name is not None:
        all_in_names.append(partition_name)

    def _body(*args):
        operands = list(args)
        if partition_name is not None:
            operands.append(bass2jax.partition_id_tensor())
        outs = bass2jax._bass_exec_p.bind(
            *operands,
            out_avals=tuple(out_avals),
            in_names=tuple(all_in_names),
            out_names=tuple(out_names),
            lowering_input_output_aliases=(),
            sim_require_finite=True,
            sim_require_nnan=True,
            nc=nc,
        )
        return tuple(outs)

    devices = jax.devices()[:NCORES]
    mesh = Mesh(np.asarray(devices), ("core",))
    n_outs = len(out_names)
    in_specs = (PartitionSpec("core"),) * (n_params + n_outs)
    out_specs = (PartitionSpec("core"),) * n_outs
    # no donation: every output element is written by the kernel, so the
    # pre-zeroed buffers don't need to be reused as outputs
    sharded = jax.jit(shard_map(_body, mesh=mesh, in_specs=in_specs,
                                out_specs=out_specs, check_rep=False),
                      keep_unused=True)
    concat_zeros = [
        np_.zeros((NCORES * z.shape[0], *z.shape[1:]), z.dtype)
        for z in zero_outs
    ]

    state = {"dev_zeros": None}

    def run(in_maps, device_inputs=None):
        if device_inputs is None:
            concat_in = [
                np_.concatenate([np_.asarray(in_maps[c][k])
                                 for c in range(NCORES)], axis=0)
                for k in in_names
            ]
        else:
            concat_in = device_inputs
        if state["dev_zeros"] is None:
            state["dev_zeros"] = [jax.device_put(
                z, jax.sharding.NamedSharding(mesh, PartitionSpec("core")))
                for z in concat_zeros]
        out_arrs = sharded(*concat_in, *state["dev_zeros"])
        jax.block_until_ready(out_arrs)
        return out_arrs

    def to_results(out_arrs):
        return [
            {name: np.asarray(out_arrs[i]).reshape(
                NCORES, *out_avals[i].shape)[c]
             for i, name in enumerate(out_names)}
            for c in range(NCORES)
        ]

    def put_inputs(in_maps):
        import jax
        sh = jax.sharding.NamedSharding(mesh, PartitionSpec("core"))
        return [jax.device_put(
            np.concatenate([np.asarray(in_maps[c][k])
                            for c in range(NCORES)], axis=0), sh)
            for k in in_names]

    _CACHED["runner"] = (run, to_results, put_inputs)
    return _CACHED["runner"]


def _run_device(in_maps):
    run, to_results, _ = _get_runner()
    return to_results(run(in_maps))


def kernel(emissions, attn_mask, tags, start_transitions, end_transitions,
           transitions):
    emissions = np.asarray(emissions, np.float32)
    tags = np.asarray(tags)
    start = np.asarray(start_transitions, np.float32)
    end = np.asarray(end_transitions, np.float32)
    trans = np.asarray(transitions, np.float32)
    # attn_mask is all ones per the input spec; the kernel relies on that.

    in_maps = _prep_core_inputs(emissions, start, trans)
    results = _run_device(in_maps)
    decode, loss = _host_postprocess(results, emissions, tags, start, end,
                                     trans)
    return decode, loss


# ---------------------------------------------------------------------------
# numpy replica of the device program, for validation without hardware
def _device_replica(in_map):
    s0 = in_map["s0"]; u0 = in_map["u0"]
    ew = in_map["ew"]; em_b = in_map["em_b"]; em_k = in_map["em_k"]
    NCH = T // CHUNK
    emb = em_b.reshape(NCH, B, CHUNK, K).transpose(0, 2, 1, 3) \
        .reshape(T, B, K)
    emk = em_k.reshape(NCH, K, CHUNK, B).transpose(0, 2, 1, 3) \
        .reshape(T, K, B)
    S = s0.copy(); u = u0.copy()
    top1 = S.max(1, keepdims=True)
    straj = np.zeros((T - 1, B, K), np.float32)
    sship = np.zeros((ND, K), np.float32)
    trans = in_map["_trans"]
    for t in range(1, T):
        # viterbi max (direct; the chained-delta fp path differs by ~3e-6)
        M = (S[:, :, None] + trans[None]).max(1)        # [B, K(j)]
        S = (M - trans[K - 1][None, :] - top1) + emb[t]
        top1 = S.max(1, keepdims=True)
        straj[t - 1] = S
        # forward replica
        P = ew.T @ u                       # [K+1, B]
        expem = np.exp(emk[t])
        if t % DIV == 0:
            d = t // DIV
            sship[d - 1] = P[K]
            rinv = 1.0 / P[K]
            u = (P[0:K] * expem) * rinv[None, :]
        else:
            u = P[0:K] * expem
    return {"s_traj": straj, "ufin": u, "s_ship": sship.reshape(1, ND * K)}


def _kernel_replica(emissions, attn_mask, tags, start_transitions,
                    end_transitions, transitions):
    """Full-numpy emulation of kernel() — for numerics validation."""
    emissions = np.asarray(emissions, np.float32)
    start = np.asarray(start_transitions, np.float32)
    end = np.asarray(end_transitions, np.float32)
    trans = np.asarray(transitions, np.float32)
    in_maps = _prep_core_inputs(emissions, start, trans)
    for m in in_maps:
        m["_trans"] = trans
    results = [_device_replica(m) for m in in_maps]
    return _host_postprocess(results, emissions, np.asarray(tags), start, end,
                             trans)


# revision 12
# speedup vs baseline: 99.6701x; 1.0481x over previous
"""CRF cell (Viterbi decode + forward log-partition + gold-path score) on 8
Trainium NeuronCores.

Problem: B=512, T=1024, K=64; mask is all-ones (per input spec).
Output: (decode int32 [B,T], loss f32 [B]).

Strategy (data-parallel over B, 64 sequences per core):
  Viterbi  — per step, the exact 64-way max over predecessors is ONE DVE
             tensor_tensor_scan over free dim (j,i) with a precomputed
             delta table:  g[j,i] = max(g[j,i-1] + (t[i-1,j]-t[i,j]), S[i])
             so g[j,63] + t[63,j] = max_i(S[i] + t[i,j]).  delta[j,0]=-1e9
             resets the chain at each j group.  Scores (renormalized by the
             per-sequence max) stream to DRAM; the host recovers the argmax
             backpointers only along the decoded paths.
  Forward  — scaled-probability domain on the TensorEngine:
             P = [exp(trans) | ones]^T @ u  (PSUM row 64 = column sum s),
             u <- P[0:64] * exp(em_t), divided by a lagged column sum every
             DIV steps; the s values stream out and the host sums their logs.
  Numerator— O(B*T) gathers, done on host.
"""
import os
import sys
import numpy as np

sys.path.insert(0, "/opt/trn_rl_repo")

B_FULL, T, K = 512, 1024, 64
NCORES = 8
B = B_FULL // NCORES          # 64 sequences per core
CHUNK = 16                    # time steps per emissions DMA
DIV = 8                       # forward renormalization cadence
ND = (T - 1) // DIV           # number of division points (t = DIV, 2*DIV, ...)
RNORM = 8                     # viterbi renormalization cadence
JI = K * K                    # viterbi scan free size

_CACHED = {}


def _build_nc():
    import concourse.bass as bass
    import concourse.bacc as bacc
    import concourse.mybir as mybir
    from concourse.tile import TileContext

    fp = mybir.dt.float32
    nc = bacc.Bacc("TRN2", target_bir_lowering=False, debug=False,
                   num_devices=NCORES)

    NCH = T // CHUNK
    # inputs (per core)
    s0 = nc.declare_dram_parameter("s0", [B, K], fp, isOutput=False)
    u0 = nc.declare_dram_parameter("u0", [K, B], fp, isOutput=False)
    delta = nc.declare_dram_parameter("delta", [B, JI], fp, isOutput=False)
    ew = nc.declare_dram_parameter("ew", [K, K + 1], fp, isOutput=False)
    ones_r = nc.declare_dram_parameter("ones_r", [1, K], fp, isOutput=False)
    # emissions, b-major with trans[63,:] pre-added: em_b[ci, b, (tt,k)]
    em_b = nc.declare_dram_parameter("em_b", [NCH, B, CHUNK * K], fp,
                                     isOutput=False)
    # emissions, k-major raw: em_k[ci, j, (tt,b)]
    em_k = nc.declare_dram_parameter("em_k", [NCH, K, CHUNK * B], fp,
                                     isOutput=False)
    # outputs
    s_traj = nc.declare_dram_parameter("s_traj", [T - 1, B, K], fp,
                                       isOutput=True)
    ufin = nc.declare_dram_parameter("ufin", [K, B], fp, isOutput=True)
    s_ship = nc.declare_dram_parameter("s_ship", [1, ND * K], fp,
                                       isOutput=True)

    with TileContext(nc) as tc:
        with (
            tc.tile_pool(name="const", bufs=1) as cpool,
            tc.tile_pool(name="state", bufs=1) as spool,
            tc.tile_pool(name="g", bufs=2) as gpool,
            tc.tile_pool(name="emb", bufs=2) as embpool,
            tc.tile_pool(name="emk", bufs=2) as emkpool,
            tc.tile_pool(name="fw", bufs=2) as fwpool,
            tc.tile_pool(name="psum", bufs=2, space="PSUM") as ppool,
            tc.tile_pool(name="psum1", bufs=1, space="PSUM") as ppool1,
        ):
            # ---- constants ----
            dtile = cpool.tile([B, JI], fp, tag="delta")
            nc.sync.dma_start(out=dtile[:], in_=delta[:])
            ewt = cpool.tile([K, K + 1], fp, tag="ew")
            nc.sync.dma_start(out=ewt[:], in_=ew[:])
            onesr = cpool.tile([1, K], fp, tag="ones")
            nc.sync.dma_start(out=onesr[:], in_=ones_r[:])

            # ---- state ----
            sA = spool.tile([B, K], fp, tag="sA")
            sB = spool.tile([B, K], fp, tag="sB")
            nc.sync.dma_start(out=sA[:], in_=s0[:])
            uA = spool.tile([K, B], fp, tag="uA")
            uB = spool.tile([K, B], fp, tag="uB")
            nc.sync.dma_start(out=uA[:], in_=u0[:])
            top1 = spool.tile([B, 1], fp, tag="top1")
            rinv = spool.tile([1, K], fp, tag="rinv")
            sshipt = spool.tile([1, ND * K], fp, tag="sship")

            # absorb multi-queue DMA waits into single vector ops
            junk = spool.tile([B, 1], fp, tag="junk")
            nc.vector.tensor_copy(junk[:1], dtile[:1, 0:1])
            nc.vector.tensor_copy(junk[:1], ewt[:1, 0:1])
            nc.vector.tensor_copy(junk[:1], sA[:1, 0:1])
            nc.vector.tensor_copy(junk[:1], uA[:1, 0:1])
            nc.vector.tensor_reduce(out=top1[:], in_=sA[:],
                                    axis=mybir.AxisListType.X,
                                    op=mybir.AluOpType.max)

            for t in range(1, T):
                ci, tt = (t // CHUNK), (t % CHUNK)
                cur, nxt = (sA, sB) if t % 2 == 1 else (sB, sA)
                ucur, unxt = (uA, uB) if t % 2 == 1 else (uB, uA)
                if tt == 0 or t == 1:
                    embt = embpool.tile([B, CHUNK * K], fp, tag="emb")
                    nc.sync.dma_start(out=embt[:], in_=em_b[ci])
                    nc.vector.tensor_copy(junk[:1], embt[:1, 0:1])
                    emkt = emkpool.tile([K, CHUNK * B], fp, tag="emk")
                    nc.sync.dma_start(out=emkt[:], in_=em_k[ci])
                    nc.vector.tensor_copy(junk[:1], emkt[:1, 0:1])

                # ---------- viterbi ----------
                g = gpool.tile([B, JI], fp, tag="g")
                g3 = bass.AP(g.tensor, g[:].offset, [g[:].ap[0], [K, K], [1, K]])
                d3 = bass.AP(dtile.tensor, dtile[:].offset,
                             [dtile[:].ap[0], [K, K], [1, K]])
                prev_b = bass.AP(cur.tensor, cur[:].offset,
                                 [cur[:].ap[0], [0, K], [1, K]])
                nc.vector.add_instruction(
                    mybir.InstTensorScalarPtr(
                        name=nc.get_next_instruction_name(),
                        is_tensor_tensor_scan=True,
                        is_scalar_tensor_tensor=True,
                        op0=mybir.AluOpType.add,
                        op1=mybir.AluOpType.max,
                        ins=[
                            nc.vector.lower_ap(d3),
                            nc.vector.lower_ap_or_imm(-1e9),
                            nc.vector.lower_ap(prev_b),
                        ],
                        outs=[nc.vector.lower_ap(g3)],
                    ))
                # S = g[:, j, 63] + em''  (em'' = em + trans[63,:] - drift)
                g63 = bass.AP(g.tensor, g[:].offset + (K - 1),
                              [g[:].ap[0], [K, K]])
                nc.vector.tensor_tensor(
                    out=nxt[:], in0=g63, in1=embt[:, tt * K:(tt + 1) * K],
                    op=mybir.AluOpType.add)
                if t % RNORM == 0:
                    nc.vector.tensor_reduce(
                        out=top1[:], in_=nxt[:], axis=mybir.AxisListType.X,
                        op=mybir.AluOpType.max)
                    t1b = bass.AP(top1.tensor, top1[:].offset,
                                  [top1[:].ap[0], [0, K]])
                    nc.vector.tensor_tensor(
                        out=nxt[:], in0=nxt[:], in1=t1b,
                        op=mybir.AluOpType.subtract)
                nc.sync.dma_start(out=s_traj[t - 1], in_=nxt[:])

                # ---------- forward ----------
                P = ppool.tile([K + 1, B], fp, tag="P")
                nc.tensor.matmul(P[:], ewt[:], ucur[:])
                expem = fwpool.tile([K, B], fp, tag="expem")
                nc.scalar.activation(
                    out=expem[:], in_=emkt[:, tt * B:(tt + 1) * B],
                    func=mybir.ActivationFunctionType.Exp)
                if t % DIV == 0:
                    d = t // DIV   # 1-based division index
                    # ship s-row; build rinv from it; broadcast via PE
                    nc.scalar.copy(out=sshipt[:, (d - 1) * K:d * K],
                                   in_=P[K:K + 1, :])
                    nc.vector.reciprocal(out=rinv[:], in_=P[K:K + 1, :])
                    rinv_rep = ppool1.tile([K, B], fp, tag="rrep")
                    nc.tensor.matmul(rinv_rep[:], onesr[:], rinv[:])
                    tmpu = fwpool.tile([K, B], fp, tag="tmpu")
                    nc.vector.tensor_tensor(
                        out=tmpu[:], in0=P[0:K, :], in1=expem[:],
                        op=mybir.AluOpType.mult)
                    nc.vector.tensor_tensor(
                        out=unxt[:], in0=tmpu[:], in1=rinv_rep[:],
                        op=mybir.AluOpType.mult)
                else:
                    nc.vector.tensor_tensor(
                        out=unxt[:], in0=P[0:K, :], in1=expem[:],
                        op=mybir.AluOpType.mult)

            last_u = uA if (T - 1) % 2 == 0 else uB
            nc.sync.dma_start(out=ufin[:], in_=last_u[:])
            nc.sync.dma_start(out=s_ship[:], in_=sshipt[:])

    nc.compile()
    return nc


def _prep_core_inputs(emissions, start, trans):
    """Per-core input dict list. emissions [B_FULL,T,K] f32."""
    NCH = T // CHUNK
    expT = np.exp(trans).astype(np.float32)
    ew = np.concatenate([expT, np.ones((K, 1), np.float32)], axis=1)  # [K,K+1]
    delta = np.empty((K, K), np.float32)          # [j, i]
    delta[:, 0] = -1e9
    delta[:, 1:] = (trans[:-1, :] - trans[1:, :]).T
    delta_b = np.broadcast_to(delta.reshape(1, JI), (B, JI)).copy()
    ones_r = np.ones((1, K), np.float32)
    # estimate the mean per-step score growth so it can be folded out of the
    # emissions (keeps |S| small between the every-RNORM renormalizations)
    Ssub = (start[None, :] + emissions[0:32, 0, :]).astype(np.float32)
    lvl0 = Ssub.max(1).mean()
    for tt in range(1, 41):
        Ssub = (Ssub[:, :, None] + trans[None]).max(1) + emissions[0:32, tt, :]
    drift = float((Ssub.max(1).mean() - lvl0) / 40.0)
    t63 = trans[K - 1] - drift                    # [K] folded into em_b

    in_maps = []
    for c in range(NCORES):
        emc = emissions[c * B:(c + 1) * B]        # [B,T,K]
        s0 = (start[None, :] + emc[:, 0, :]).astype(np.float32)
        u0 = np.exp(s0).T.copy()                  # [K,B]
        # em_b: steps 1..T-1 used; keep full T for simple indexing
        emb = (emc + t63[None, None, :]).astype(np.float32)       # [B,T,K]
        emb = emb.transpose(1, 0, 2).reshape(NCH, CHUNK, B, K) \
            .transpose(0, 2, 1, 3).reshape(NCH, B, CHUNK * K).copy()
        emk = emc.transpose(1, 2, 0).reshape(NCH, CHUNK, K, B) \
            .transpose(0, 2, 1, 3).reshape(NCH, K, CHUNK * B).copy()
        in_maps.append({"s0": s0, "u0": u0, "delta": delta_b, "ew": ew,
                        "ones_r": ones_r, "em_b": emb, "em_k": emk})
    return in_maps


def _host_postprocess(results, emissions, tags, start, end, trans):
    """Backtrack + numerator + loss from device outputs."""
    Bf = B_FULL
    # gather trajectories: S[t] for t=1..T-1 per core -> [T-1, B_FULL, K]
    straj = np.concatenate([r["s_traj"] for r in results], axis=1)
    s0_full = (start[None, :] + emissions[:, 0, :]).astype(np.float32)

    # ---- viterbi backtrack ----
    transT = np.ascontiguousarray(trans.T)        # [j, i]
    last = np.argmax(straj[T - 2] + end[None, :], axis=1).astype(np.int32)
    decode = np.empty((Bf, T), np.int32)
    decode[:, T - 1] = last
    idx = last
    for t in range(T - 2, -1, -1):
        Sprev = straj[t - 1] if t >= 1 else s0_full
        idx = np.argmax(Sprev + transT[idx], axis=1).astype(np.int32)
        decode[:, t] = idx

    # ---- logZ ----
    ufin = np.concatenate([r["ufin"] for r in results], axis=1)   # [K, B_FULL]
    # s_ship[c] is [ND, B] (B == K == 64): per-division column sums per seq
    logacc = np.concatenate(
        [np.log(r["s_ship"].reshape(ND, B)).sum(axis=0) for r in results])
    z_tail = (np.exp(end)[:, None] * ufin).sum(axis=0)            # [B_FULL]
    logz = logacc + np.log(z_tail)

    # ---- numerator (gold path score), mask all ones ----
    tg = tags.astype(np.int64)
    em_sc = np.take_along_axis(emissions, tg[:, :, None], axis=2)[:, :, 0]
    trans_sc = trans[tg[:, :-1], tg[:, 1:]]
    num = (start[tg[:, 0]] + em_sc.sum(axis=1) + trans_sc.sum(axis=1)
           + end[tg[:, -1]])

    loss = -(num - logz)
    return decode, loss.astype(np.float32)


def _get_runner():
    """Build + jit the SPMD executable once; returns a callable over in_maps.

    Modeled on bass2jax.run_bass_via_pjrt, but caches the jitted function so
    repeat calls skip retrace/recompile, and exposes device-level run for
    timing.
    """
    if "runner" in _CACHED:
        return _CACHED["runner"]
    import jax
    import numpy as np_
    from jax.sharding import Mesh, PartitionSpec
    from jax.experimental.shard_map import shard_map
    import concourse.mybir as mybir
    from concourse import bass2jax

    nc = _build_nc()
    bass2jax.install_neuronx_cc_hook()
    partition_name = (nc.partition_id_tensor.name
                      if nc.partition_id_tensor else None)

    in_names, out_names, out_avals, zero_outs = [], [], [], []
    for alloc in nc.m.functions[0].allocations:
        if not isinstance(alloc, mybir.MemoryLocationSet):
            continue
        name = alloc.memorylocations[0].name
        if alloc.kind == "ExternalInput":
            if name != partition_name:
                in_names.append(name)
        elif alloc.kind == "ExternalOutput":
            out_names.append(name)
            shape = tuple(alloc.tensor_shape)
            dtype = mybir.dt.np(alloc.dtype)
            out_avals.append(jax.core.ShapedArray(shape, dtype))
            zero_outs.append(np_.zeros(shape, dtype))
    n_params = len(in_names)
    all_in_names = list(in_names) + list(out_names)
    if partition_name is not None:
        all_in_names.append(partition_name)

    def _body(*args):
        operands = list(args)
        if partition_name is not None:
            operands.append(bass2jax.partition_id_tensor())
        outs = bass2jax._bass_exec_p.bind(
            *operands,
            out_avals=tuple(out_avals),
            in_names=tuple(all_in_names),
            out_names=tuple(out_names),
            lowering_input_output_aliases=(),
            sim_require_finite=True,
            sim_require_nnan=True,
            nc=nc,
        )
        return tuple(outs)

    devices = jax.devices()[:NCORES]
    mesh = Mesh(np.asarray(devices), ("core",))
    n_outs = len(out_names)
    in_specs = (PartitionSpec("core"),) * (n_params + n_outs)
    out_specs = (PartitionSpec("core"),) * n_outs
    # no donation: every output element is written by the kernel, so the
    # pre-zeroed buffers don't need to be reused as outputs
    sharded = jax.jit(shard_map(_body, mesh=mesh, in_specs=in_specs,
                                out_specs=out_specs, check_rep=False),
                      keep_unused=True)
    concat_zeros = [
        np_.zeros((NCORES * z.shape[0], *z.shape[1:]), z.dtype)
        for z in zero_outs
    ]

    state = {"dev_zeros": None}

    def run(in_maps, device_inputs=None):
        if device_inputs is None:
            concat_in = [
                np_.concatenate([np_.asarray(in_maps[c][k])
                                 for c in range(NCORES)], axis=0)
                for k in in_names
            ]
        else:
            concat_in = device_inputs
        if state["dev_zeros"] is None:
            state["dev_zeros"] = [jax.device_put(
                z, jax.sharding.NamedSharding(mesh, PartitionSpec("core")))
                for z in concat_zeros]
        out_arrs = sharded(*concat_in, *state["dev_zeros"])
        jax.block_until_ready(out_arrs)
        return out_arrs

    def to_results(out_arrs):
        return [
            {name: np.asarray(out_arrs[i]).reshape(
                NCORES, *out_avals[i].shape)[c]
             for i, name in enumerate(out_names)}
            for c in range(NCORES)
        ]

    def put_inputs(in_maps):
        import jax
        sh = jax.sharding.NamedSharding(mesh, PartitionSpec("core"))
        return [jax.device_put(
            np.concatenate([np.asarray(in_maps[c][k])
                            for c in range(NCORES)], axis=0), sh)
            for k in in_names]

    _CACHED["runner"] = (run, to_results, put_inputs)
    return _CACHED["runner"]


def _run_device(in_maps):
    run, to_results, _ = _get_runner()
    return to_results(run(in_maps))


def kernel(emissions, attn_mask, tags, start_transitions, end_transitions,
           transitions):
    emissions = np.asarray(emissions, np.float32)
    tags = np.asarray(tags)
    start = np.asarray(start_transitions, np.float32)
    end = np.asarray(end_transitions, np.float32)
    trans = np.asarray(transitions, np.float32)
    # attn_mask is all ones per the input spec; the kernel relies on that.

    in_maps = _prep_core_inputs(emissions, start, trans)
    results = _run_device(in_maps)
    decode, loss = _host_postprocess(results, emissions, tags, start, end,
                                     trans)
    return decode, loss


# ---------------------------------------------------------------------------
# numpy replica of the device program, for validation without hardware
def _device_replica(in_map):
    s0 = in_map["s0"]; u0 = in_map["u0"]
    ew = in_map["ew"]; em_b = in_map["em_b"]; em_k = in_map["em_k"]
    NCH = T // CHUNK
    emb = em_b.reshape(NCH, B, CHUNK, K).transpose(0, 2, 1, 3) \
        .reshape(T, B, K)
    emk = em_k.reshape(NCH, K, CHUNK, B).transpose(0, 2, 1, 3) \
        .reshape(T, K, B)
    S = s0.copy(); u = u0.copy()
    straj = np.zeros((T - 1, B, K), np.float32)
    sship = np.zeros((ND, K), np.float32)
    trans = in_map["_trans"]
    for t in range(1, T):
        # viterbi max (direct; the chained-delta fp path differs by ~3e-6)
        M = (S[:, :, None] + trans[None]).max(1)        # [B, K(j)]
        S = (M - trans[K - 1][None, :]) + emb[t]        # emb holds em + t63 - drift
        if t % RNORM == 0:
            S = S - S.max(1, keepdims=True)
        straj[t - 1] = S
        # forward replica
        P = ew.T @ u                       # [K+1, B]
        expem = np.exp(emk[t])
        if t % DIV == 0:
            d = t // DIV
            sship[d - 1] = P[K]
            rinv = 1.0 / P[K]
            u = (P[0:K] * expem) * rinv[None, :]
        else:
            u = P[0:K] * expem
    return {"s_traj": straj, "ufin": u, "s_ship": sship.reshape(1, ND * K)}


def _kernel_replica(emissions, attn_mask, tags, start_transitions,
                    end_transitions, transitions):
    """Full-numpy emulation of kernel() — for numerics validation."""
    emissions = np.asarray(emissions, np.float32)
    start = np.asarray(start_transitions, np.float32)
    end = np.asarray(end_transitions, np.float32)
    trans = np.asarray(transitions, np.float32)
    in_maps = _prep_core_inputs(emissions, start, trans)
    for m in in_maps:
        m["_trans"] = trans
    results = [_device_replica(m) for m in in_maps]
    return _host_postprocess(results, emissions, np.asarray(tags), start, end,
                             trans)


# revision 19
# speedup vs baseline: 112.1651x; 1.1254x over previous
"""CRF cell (Viterbi decode + forward log-partition + gold-path score) on 8
Trainium NeuronCores.

Problem: B=512, T=1024, K=64; mask is all-ones (per input spec).
Output: (decode int32 [B,T], loss f32 [B]).

Strategy (data-parallel over B, 64 sequences per core):
  Viterbi  — per step, the exact 64-way max over predecessors is ONE DVE
             tensor_tensor_scan over free dim (j,i) with a precomputed
             delta table:  g[j,i] = max(g[j,i-1] + (t[i-1,j]-t[i,j]), S[i])
             so g[j,63] + t[63,j] = max_i(S[i] + t[i,j]).  delta[j,0]=-1e9
             resets the chain at each j group.  Scores (renormalized by the
             per-sequence max) stream to DRAM; the host recovers the argmax
             backpointers only along the decoded paths.
  Forward  — scaled-probability domain on the TensorEngine:
             P = [exp(trans) | ones]^T @ u  (PSUM row 64 = column sum s),
             u <- P[0:64] * exp(em_t), divided by a lagged column sum every
             DIV steps; the s values stream out and the host sums their logs.
  Numerator— O(B*T) gathers, done on host.
"""
import os
import sys
import numpy as np

sys.path.insert(0, "/opt/trn_rl_repo")

B_FULL, T, K = 512, 1024, 64
NCORES = 8
B = B_FULL // NCORES          # 64 sequences per core
CHUNK = 16                    # time steps per emissions DMA
DIV = 8                       # forward renormalization cadence
ND = (T - 1) // DIV           # number of division points (t = DIV, 2*DIV, ...)
RNORM = 8                     # viterbi renormalization cadence
JI = K * K                    # viterbi scan free size

_CACHED = {}
_FLAGS = {"scan": True, "dma": True, "fwd": True, "nsteps": None}


def _build_nc():
    import concourse.bass as bass
    import concourse.bacc as bacc
    import concourse.mybir as mybir
    from concourse.tile import TileContext

    fp = mybir.dt.float32
    nc = bacc.Bacc("TRN2", target_bir_lowering=False, debug=False,
                   num_devices=NCORES)

    NCH = T // CHUNK
    JL = K // 2                   # 32 j-columns per partition half
    JI2 = JL * K                  # scan free size (2048)
    # inputs (per core)
    s0 = nc.declare_dram_parameter("s0", [2 * B, JL], fp, isOutput=False)
    u0 = nc.declare_dram_parameter("u0", [K, B], fp, isOutput=False)
    delta = nc.declare_dram_parameter("delta", [2 * B, JI2], fp, isOutput=False)
    lsel = nc.declare_dram_parameter("lsel", [2 * B, 2 * B], fp, isOutput=False)
    hsel = nc.declare_dram_parameter("hsel", [2 * B, 2 * B], fp, isOutput=False)
    ew = nc.declare_dram_parameter("ew", [K, K + 1], fp, isOutput=False)
    ones_r = nc.declare_dram_parameter("ones_r", [1, K], fp, isOutput=False)
    # emissions, (jh,b)-major with trans[63,:]-drift folded: [NCH, 128, CHUNK*JL]
    em_b = nc.declare_dram_parameter("em_b", [NCH, 2 * B, CHUNK * JL], fp,
                                     isOutput=False)
    # emissions, k-major raw: em_k[ci, j, (tt,b)]
    em_k = nc.declare_dram_parameter("em_k", [NCH, K, CHUNK * B], fp,
                                     isOutput=False)
    # outputs
    s_traj = nc.declare_dram_parameter("s_traj", [T - 1, 2 * B, JL], fp,
                                       isOutput=True)
    ufin = nc.declare_dram_parameter("ufin", [K, B], fp, isOutput=True)
    s_ship = nc.declare_dram_parameter("s_ship", [1, ND * B], fp,
                                       isOutput=True)

    with TileContext(nc) as tc:
        with (
            tc.tile_pool(name="const", bufs=1) as cpool,
            tc.tile_pool(name="state", bufs=1) as spool,
            tc.tile_pool(name="g", bufs=2) as gpool,
            tc.tile_pool(name="emb", bufs=2) as embpool,
            tc.tile_pool(name="emk", bufs=2) as emkpool,
            tc.tile_pool(name="fw", bufs=2) as fwpool,
            tc.tile_pool(name="psum", bufs=2, space="PSUM") as ppool,
            tc.tile_pool(name="psum1", bufs=1, space="PSUM") as ppool1,
            tc.tile_pool(name="psum2", bufs=1, space="PSUM") as ppool2,
        ):
            # ---- constants ----
            dtile = cpool.tile([2 * B, JI2], fp, tag="delta")
            nc.sync.dma_start(out=dtile[:], in_=delta[:])
            lselt = cpool.tile([2 * B, 2 * B], fp, tag="lsel")
            nc.sync.dma_start(out=lselt[:], in_=lsel[:])
            hselt = cpool.tile([2 * B, 2 * B], fp, tag="hsel")
            nc.sync.dma_start(out=hselt[:], in_=hsel[:])
            ewt = cpool.tile([K, K + 1], fp, tag="ew")
            nc.sync.dma_start(out=ewt[:], in_=ew[:])
            onesr = cpool.tile([1, K], fp, tag="ones")
            nc.sync.dma_start(out=onesr[:], in_=ones_r[:])

            # ---- state ----
            supd = spool.tile([2 * B, JL], fp, tag="supd")
            nc.sync.dma_start(out=supd[:], in_=s0[:])
            uA = spool.tile([K, B], fp, tag="uA")
            uB = spool.tile([K, B], fp, tag="uB")
            nc.sync.dma_start(out=uA[:], in_=u0[:])
            top1 = spool.tile([2 * B, 1], fp, tag="top1")
            rinv = spool.tile([1, K], fp, tag="rinv")
            sshipt = spool.tile([1, ND * B], fp, tag="sship")
            s2p = ppool2.tile([2 * B, K], fp, tag="s2p")

            # absorb multi-queue DMA waits into single vector ops
            junk = spool.tile([2 * B, 1], fp, tag="junk")
            for tt_ in (dtile, lselt, hselt, ewt, supd, uA):
                nc.vector.tensor_copy(junk[:1], tt_[:1, 0:1])

            T_ = _FLAGS["nsteps"] or T
            for t in range(1, T_):
                ci, tt = (t // CHUNK), (t % CHUNK)
                ucur, unxt = (uA, uB) if t % 2 == 1 else (uB, uA)
                if tt == 0 or t == 1:
                    embt = embpool.tile([2 * B, CHUNK * JL], fp, tag="emb")
                    nc.sync.dma_start(out=embt[:], in_=em_b[ci])
                    nc.vector.tensor_copy(junk[:1], embt[:1, 0:1])
                    emkt = emkpool.tile([K, CHUNK * B], fp, tag="emk")
                    nc.sync.dma_start(out=emkt[:], in_=em_k[ci])
                    nc.vector.tensor_copy(junk[:1], emkt[:1, 0:1])

                # ---------- viterbi ----------
                if _FLAGS["scan"]:
                    # replicate supd [128,32] -> s2p [128,64] via selector mms
                    nc.tensor.matmul(s2p[:, 0:JL], lselt[:], supd[:])
                    nc.tensor.matmul(s2p[:, JL:K], hselt[:], supd[:])
                    if t % RNORM == 1 and t > 1:
                        nc.vector.tensor_reduce(
                            out=top1[:], in_=s2p[:], axis=mybir.AxisListType.X,
                            op=mybir.AluOpType.max)
                        t1b = bass.AP(top1.tensor, top1[:].offset,
                                      [top1[:].ap[0], [0, K]])
                        nc.vector.tensor_tensor(
                            out=s2p[:], in0=s2p[:], in1=t1b,
                            op=mybir.AluOpType.subtract)
                    g = gpool.tile([2 * B, JI2], fp, tag="g")
                    g3 = bass.AP(g.tensor, g[:].offset,
                                 [g[:].ap[0], [K, JL], [1, K]])
                    d3 = bass.AP(dtile.tensor, dtile[:].offset,
                                 [dtile[:].ap[0], [K, JL], [1, K]])
                    prev_b = bass.AP(s2p.tensor, s2p[:].offset,
                                     [s2p[:].ap[0], [0, JL], [1, K]])
                    nc.vector.add_instruction(
                        mybir.InstTensorScalarPtr(
                            name=nc.get_next_instruction_name(),
                            is_tensor_tensor_scan=True,
                            is_scalar_tensor_tensor=True,
                            op0=mybir.AluOpType.add,
                            op1=mybir.AluOpType.max,
                            ins=[
                                nc.vector.lower_ap(d3),
                                nc.vector.lower_ap_or_imm(-1e9),
                                nc.vector.lower_ap(prev_b),
                            ],
                            outs=[nc.vector.lower_ap(g3)],
                        ))
                    # S = g[:, j, 63] + em''  (em'' = em + trans[63,:] - drift)
                    g63 = bass.AP(g.tensor, g[:].offset + (K - 1),
                                  [g[:].ap[0], [K, JL]])
                else:
                    g63 = supd[:]
                nc.vector.tensor_tensor(
                    out=supd[:], in0=g63, in1=embt[:, tt * JL:(tt + 1) * JL],
                    op=mybir.AluOpType.add)
                if _FLAGS["dma"]:
                    nc.sync.dma_start(out=s_traj[t - 1], in_=supd[:])

                # ---------- forward ----------
                if not _FLAGS["fwd"]:
                    continue
                P = ppool.tile([K + 1, B], fp, tag="P")
                nc.tensor.matmul(P[:], ewt[:], ucur[:])
                expem = fwpool.tile([K, B], fp, tag="expem")
                nc.scalar.activation(
                    out=expem[:], in_=emkt[:, tt * B:(tt + 1) * B],
                    func=mybir.ActivationFunctionType.Exp)
                if t % DIV == 0:
                    d = t // DIV   # 1-based division index
                    # ship s-row; build rinv from it; broadcast via PE
                    nc.scalar.copy(out=sshipt[:, (d - 1) * B:d * B],
                                   in_=P[K:K + 1, :])
                    nc.vector.reciprocal(out=rinv[:], in_=P[K:K + 1, :])
                    rinv_rep = ppool1.tile([K, B], fp, tag="rrep")
                    nc.tensor.matmul(rinv_rep[:], onesr[:], rinv[:])
                    tmpu = fwpool.tile([K, B], fp, tag="tmpu")
                    nc.vector.tensor_tensor(
                        out=tmpu[:], in0=P[0:K, :], in1=expem[:],
                        op=mybir.AluOpType.mult)
                    nc.vector.tensor_tensor(
                        out=unxt[:], in0=tmpu[:], in1=rinv_rep[:],
                        op=mybir.AluOpType.mult)
                else:
                    nc.vector.tensor_tensor(
                        out=unxt[:], in0=P[0:K, :], in1=expem[:],
                        op=mybir.AluOpType.mult)

            last_u = uA if (T - 1) % 2 == 0 else uB
            nc.sync.dma_start(out=ufin[:], in_=last_u[:])
            nc.sync.dma_start(out=s_ship[:], in_=sshipt[:])

    nc.compile()
    return nc


def _prep_core_inputs(emissions, start, trans):
    """Per-core input dict list. emissions [B_FULL,T,K] f32."""
    NCH = T // CHUNK
    JL = K // 2
    expT = np.exp(trans).astype(np.float32)
    ew = np.concatenate([expT, np.ones((K, 1), np.float32)], axis=1)  # [K,K+1]
    delta = np.empty((K, K), np.float32)          # [j, i]
    delta[:, 0] = -1e9
    delta[:, 1:] = (trans[:-1, :] - trans[1:, :]).T
    # (jh,b)-partition layout: row jh*B+b holds j = jh*JL + j_lo
    delta2 = np.empty((2 * B, JL * K), np.float32)
    for jh in range(2):
        blk = delta[jh * JL:(jh + 1) * JL].reshape(JL * K)
        delta2[jh * B:(jh + 1) * B] = blk[None, :]
    ones_r = np.ones((1, K), np.float32)
    # selector matrices for the PE state replication
    lsel = np.zeros((2 * B, 2 * B), np.float32)
    hsel = np.zeros((2 * B, 2 * B), np.float32)
    p = np.arange(2 * B)
    lsel[p % B, p] = 1.0
    hsel[B + (p % B), p] = 1.0
    # estimate the mean per-step score growth so it can be folded out of the
    # emissions (keeps |S| small between the every-RNORM renormalizations)
    Ssub = (start[None, :] + emissions[0:32, 0, :]).astype(np.float32)
    lvl0 = Ssub.max(1).mean()
    for tt in range(1, 41):
        Ssub = (Ssub[:, :, None] + trans[None]).max(1) + emissions[0:32, tt, :]
    drift = float((Ssub.max(1).mean() - lvl0) / 40.0)
    t63 = trans[K - 1] - drift                    # [K] folded into em_b

    in_maps = []
    for c in range(NCORES):
        emc = emissions[c * B:(c + 1) * B]        # [B,T,K]
        s0 = (start[None, :] + emc[:, 0, :]).astype(np.float32)
        s0_2 = s0.reshape(B, 2, JL).transpose(1, 0, 2).reshape(2 * B, JL).copy()
        u0 = np.exp(s0).T.copy()                  # [K,B]
        emb = (emc + t63[None, None, :]).astype(np.float32)       # [B,T,K]
        # em_b2[ci, jh*B+b, tt*JL + jl] = emb[b, ci*CHUNK+tt, jh*JL+jl]
        emb2 = emb.transpose(1, 0, 2).reshape(NCH, CHUNK, B, 2, JL) \
            .transpose(0, 3, 2, 1, 4).reshape(NCH, 2 * B, CHUNK * JL).copy()
        emk = emc.transpose(1, 2, 0).reshape(NCH, CHUNK, K, B) \
            .transpose(0, 2, 1, 3).reshape(NCH, K, CHUNK * B).copy()
        in_maps.append({"s0": s0_2, "u0": u0, "delta": delta2, "ew": ew,
                        "lsel": lsel, "hsel": hsel,
                        "ones_r": ones_r, "em_b": emb2, "em_k": emk})
    return in_maps


def _host_postprocess(results, emissions, tags, start, end, trans):
    """Backtrack + numerator + loss from device outputs."""
    Bf = B_FULL
    # gather trajectories: S[t] for t=1..T-1 per core -> [T-1, B_FULL, K]
    JL = K // 2
    straj = np.concatenate(
        [r["s_traj"].reshape(T - 1, 2, B, JL).transpose(0, 2, 1, 3)
         .reshape(T - 1, B, K) for r in results], axis=1)
    s0_full = (start[None, :] + emissions[:, 0, :]).astype(np.float32)

    # ---- viterbi backtrack ----
    transT = np.ascontiguousarray(trans.T)        # [j, i]
    last = np.argmax(straj[T - 2] + end[None, :], axis=1).astype(np.int32)
    decode = np.empty((Bf, T), np.int32)
    decode[:, T - 1] = last
    idx = last
    for t in range(T - 2, -1, -1):
        Sprev = straj[t - 1] if t >= 1 else s0_full
        idx = np.argmax(Sprev + transT[idx], axis=1).astype(np.int32)
        decode[:, t] = idx

    # ---- logZ ----
    ufin = np.concatenate([r["ufin"] for r in results], axis=1)   # [K, B_FULL]
    # s_ship[c] is [ND, B] (B == K == 64): per-division column sums per seq
    logacc = np.concatenate(
        [np.log(r["s_ship"].reshape(ND, B)).sum(axis=0) for r in results])
    z_tail = (np.exp(end)[:, None] * ufin).sum(axis=0)            # [B_FULL]
    logz = logacc + np.log(z_tail)

    # ---- numerator (gold path score), mask all ones ----
    tg = tags.astype(np.int64)
    em_sc = np.take_along_axis(emissions, tg[:, :, None], axis=2)[:, :, 0]
    trans_sc = trans[tg[:, :-1], tg[:, 1:]]
    num = (start[tg[:, 0]] + em_sc.sum(axis=1) + trans_sc.sum(axis=1)
           + end[tg[:, -1]])

    loss = -(num - logz)
    return decode, loss.astype(np.float32)


def _get_runner():
    """Build + jit the SPMD executable once; returns a callable over in_maps.

    Modeled on bass2jax.run_bass_via_pjrt, but caches the jitted function so
    repeat calls skip retrace/recompile, and exposes device-level run for
    timing.
    """
    if "runner" in _CACHED:
        return _CACHED["runner"]
    import jax
    import numpy as np_
    from jax.sharding import Mesh, PartitionSpec
    from jax.experimental.shard_map import shard_map
    import concourse.mybir as mybir
    from concourse import bass2jax

    nc = _build_nc()
    bass2jax.install_neuronx_cc_hook()
    partition_name = (nc.partition_id_tensor.name
                      if nc.partition_id_tensor else None)

    in_names, out_names, out_avals, zero_outs = [], [], [], []
    for alloc in nc.m.functions[0].allocations:
        if not isinstance(alloc, mybir.MemoryLocationSet):
            continue
        name = alloc.memorylocations[0].name
        if alloc.kind == "ExternalInput":
            if name != partition_name:
                in_names.append(name)
        elif alloc.kind == "ExternalOutput":
            out_names.append(name)
            shape = tuple(alloc.tensor_shape)
            dtype = mybir.dt.np(alloc.dtype)
            out_avals.append(jax.core.ShapedArray(shape, dtype))
            zero_outs.append(np_.zeros(shape, dtype))
    n_params = len(in_names)
    all_in_names = list(in_names) + list(out_names)
    if partition_name is not None:
        all_in_names.append(partition_name)

    def _body(*args):
        operands = list(args)
        if partition_name is not None:
            operands.append(bass2jax.partition_id_tensor())
        outs = bass2jax._bass_exec_p.bind(
            *operands,
            out_avals=tuple(out_avals),
            in_names=tuple(all_in_names),
            out_names=tuple(out_names),
            lowering_input_output_aliases=(),
            sim_require_finite=True,
            sim_require_nnan=True,
            nc=nc,
        )
        return tuple(outs)

    devices = jax.devices()[:NCORES]
    mesh = Mesh(np.asarray(devices), ("core",))
    n_outs = len(out_names)
    in_specs = (PartitionSpec("core"),) * (n_params + n_outs)
    out_specs = (PartitionSpec("core"),) * n_outs
    # no donation: every output element is written by the kernel, so the
    # pre-zeroed buffers don't need to be reused as outputs
    sharded = jax.jit(shard_map(_body, mesh=mesh, in_specs=in_specs,
                                out_specs=out_specs, check_rep=False),
                      keep_unused=True)
    concat_zeros = [
        np_.zeros((NCORES * z.shape[0], *z.shape[1:]), z.dtype)
        for z in zero_outs
    ]

    state = {"dev_zeros": None}

    def run(in_maps, device_inputs=None):
        if device_inputs is None:
            concat_in = [
                np_.concatenate([np_.asarray(in_maps[c][k])
                                 for c in range(NCORES)], axis=0)
                for k in in_names
            ]
        else:
            concat_in = device_inputs
        if state["dev_zeros"] is None:
            state["dev_zeros"] = [jax.device_put(
                z, jax.sharding.NamedSharding(mesh, PartitionSpec("core")))
                for z in concat_zeros]
        out_arrs = sharded(*concat_in, *state["dev_zeros"])
        jax.block_until_ready(out_arrs)
        return out_arrs

    def to_results(out_arrs):
        return [
            {name: np.asarray(out_arrs[i]).reshape(
                NCORES, *out_avals[i].shape)[c]
             for i, name in enumerate(out_names)}
            for c in range(NCORES)
        ]

    def put_inputs(in_maps):
        import jax
        sh = jax.sharding.NamedSharding(mesh, PartitionSpec("core"))
        return [jax.device_put(
            np.concatenate([np.asarray(in_maps[c][k])
                            for c in range(NCORES)], axis=0), sh)
            for k in in_names]

    _CACHED["runner"] = (run, to_results, put_inputs)
    return _CACHED["runner"]


def _run_device(in_maps):
    run, to_results, _ = _get_runner()
    return to_results(run(in_maps))


def kernel(emissions, attn_mask, tags, start_transitions, end_transitions,
           transitions):
    emissions = np.asarray(emissions, np.float32)
    tags = np.asarray(tags)
    start = np.asarray(start_transitions, np.float32)
    end = np.asarray(end_transitions, np.float32)
    trans = np.asarray(transitions, np.float32)
    # attn_mask is all ones per the input spec; the kernel relies on that.

    in_maps = _prep_core_inputs(emissions, start, trans)
    results = _run_device(in_maps)
    decode, loss = _host_postprocess(results, emissions, tags, start, end,
                                     trans)
    return decode, loss


# ---------------------------------------------------------------------------
# numpy replica of the device program, for validation without hardware
def _device_replica(in_map):
    ew = in_map["ew"]; em_b = in_map["em_b"]; em_k = in_map["em_k"]
    NCH = T // CHUNK
    JL = K // 2
    s0 = in_map["s0"].reshape(2, B, JL).transpose(1, 0, 2).reshape(B, K)
    u0 = in_map["u0"]
    emb = em_b.reshape(NCH, 2, B, CHUNK, JL).transpose(0, 3, 2, 1, 4) \
        .reshape(T, B, K)
    emk = em_k.reshape(NCH, K, CHUNK, B).transpose(0, 2, 1, 3) \
        .reshape(T, K, B)
    S = s0.copy(); u = u0.copy()
    straj = np.zeros((T - 1, B, K), np.float32)
    sship = np.zeros((ND, B), np.float32)
    trans = in_map["_trans"]
    for t in range(1, T):
        if t % RNORM == 1 and t > 1:
            S = S - S.max(1, keepdims=True)
        # viterbi max (direct; the chained-delta fp path differs by ~3e-6)
        M = (S[:, :, None] + trans[None]).max(1)        # [B, K(j)]
        S = (M - trans[K - 1][None, :]) + emb[t]        # emb holds em + t63 - drift
        straj[t - 1] = S
        # forward replica
        P = ew.T @ u                       # [K+1, B]
        expem = np.exp(emk[t])
        if t % DIV == 0:
            d = t // DIV
            sship[d - 1] = P[K]
            rinv = 1.0 / P[K]
            u = (P[0:K] * expem) * rinv[None, :]
        else:
            u = P[0:K] * expem
    return {"s_traj": straj.reshape(T - 1, B, 2, JL).transpose(0, 2, 1, 3)
            .reshape(T - 1, 2 * B, JL),
            "ufin": u, "s_ship": sship.reshape(1, ND * B)}


def _kernel_replica(emissions, attn_mask, tags, start_transitions,
                    end_transitions, transitions):
    """Full-numpy emulation of kernel() — for numerics validation."""
    emissions = np.asarray(emissions, np.float32)
    start = np.asarray(start_transitions, np.float32)
    end = np.asarray(end_transitions, np.float32)
    trans = np.asarray(transitions, np.float32)
    in_maps = _prep_core_inputs(emissions, start, trans)
    for m in in_maps:
        m["_trans"] = trans
    results = [_device_replica(m) for m in in_maps]
    return _host_postprocess(results, emissions, np.asarray(tags), start, end,
                             trans)
